# revision 16
# baseline (speedup 1.0000x reference)
"""Trainium2 (Bass/Tile) kernel for nn_BoxGauss: gaussian-box-masked MSE loss.

reference semantics (per pyramid level l with preds/trues [B, C, S, S]):
    m      = gauss_mask(bboxes, batch_idx, S, B)        # [B, S, S]
    n_pos  = C * sum(m)
    ssq    = sum((m[:, None] * (pred - true)) ** 2)
    total += ssq / n_pos
  output = total / n_levels                              # scalar f32

Strategy (data-parallel over 8 NeuronCores, 2 images per core):
  * The tiny mask m (built from 256 boxes) is computed on the host in
    fp32, mirroring the reference op-for-op; m**2 is shipped per-core,
    pre-arranged to the on-chip chunk layout (a few tens of KB).
  * Each core streams its 2 images of pred/true per level from HBM
    (~22.9 MB/core, the memory-bound bulk of the problem):
        DVE:  d = p - t                     (fp32 in, bf16 out)
        ACT:  e = d^2                       (bf16, Square is spline-exact)
        PE :  colsq[px_chunk] = ones^T-contraction over channels,
              i.e. matmul(lhsT=e[K=C_tile, M=px], rhs=ones[K,1]) -> PSUM
              accumulated over C tiles; pixels land on PSUM partitions.
        DVE:  tensor_tensor_reduce(psum * m^2) -> per-unit partial sums
  * Each core returns stats [128, 8]; host reduces the 8x tiny partials
    and applies the n_pos normalizers (all tiny scalar math).

Self-contained: shapes/sharding hardcoded for the
  y_pred0/1/2 [16,128,80,80]/[16,256,40,40]/[16,512,20,20] problem.
"""

import numpy as np

N_CORES = 8
B = 16
IPC = B // N_CORES  # images per core
STD = 2.0

# (C, S) per level
LEVELS = [(128, 80), (256, 40), (512, 20)]

_PROG_CACHE = {}
LAST_RESULTS = None  # BassKernelResults of the most recent device run


# --------------------------------------------------------------------------
# host-side mask (mirrors reference._gauss_mask in fp32 numpy)
# --------------------------------------------------------------------------
def _gauss_mask_np(bboxes, batch_idx, S):
    f32 = np.float32
    bb = np.asarray(bboxes, dtype=f32)
    g = np.floor(bb * f32(S)).astype(np.int32)
    xc, yc, w, h = g[:, 0], g[:, 1], g[:, 2], g[:, 3]
    xl = np.maximum(xc - w // 2, 0)
    xr = np.minimum(xc + w // 2, S - 1)
    yt = np.maximum(yc - h // 2, 0)
    yd = np.minimum(yc + h // 2, S - 1)
    width = (xr - xl + 1).astype(f32)
    height = (yd - yt + 1).astype(f32)
    ax = np.arange(S, dtype=f32)
    xcf = xc.astype(f32)
    ycf = yc.astype(f32)
    tx = (ax[None, :] - xcf[:, None]) ** 2 / (
        f32(STD * STD) * (width[:, None] / f32(2)) ** 2
    )
    ty = (ax[None, :] - ycf[:, None]) ** 2 / (
        f32(STD * STD) * (height[:, None] / f32(2)) ** 2
    )
    gauss = np.exp(-(tx[:, None, :] + ty[:, :, None]))  # [N, S, S] f32
    ix = (ax[None, :] >= xl[:, None]) & (ax[None, :] <= xr[:, None])
    iy = (ax[None, :] >= yt[:, None]) & (ax[None, :] <= yd[:, None])
    inbox = ix[:, None, :] & iy[:, :, None]
    gauss = np.where(inbox, gauss, f32(0))
    m = np.zeros((B, S, S), dtype=f32)
    bi = np.asarray(batch_idx)
    for n in range(bb.shape[0]):
        np.maximum(m[bi[n]], gauss[n], out=m[bi[n]])
    return m


# --------------------------------------------------------------------------
# device program (SPMD: same program on all 8 cores, per-core inputs)
# --------------------------------------------------------------------------
def build_program():
    if "nc" in _PROG_CACHE:
        return _PROG_CACHE["nc"]

    from contextlib import ExitStack

    import concourse.tile as tile
    from concourse import bacc, mybir

    f32 = mybir.dt.float32
    bf16 = mybir.dt.bfloat16
    Alu = mybir.AluOpType

    nc = bacc.Bacc("TRN2", target_bir_lowering=False, debug=False)

    p0 = nc.dram_tensor("p0", [IPC, 128, 6400], f32, kind="ExternalInput").ap()
    t0 = nc.dram_tensor("t0", [IPC, 128, 6400], f32, kind="ExternalInput").ap()
    p1 = nc.dram_tensor("p1", [IPC, 256, 1600], f32, kind="ExternalInput").ap()
    t1 = nc.dram_tensor("t1", [IPC, 256, 1600], f32, kind="ExternalInput").ap()
    p2 = nc.dram_tensor("p2", [IPC, 512, 400], f32, kind="ExternalInput").ap()
    t2 = nc.dram_tensor("t2", [IPC, 512, 400], f32, kind="ExternalInput").ap()
    msq0 = nc.dram_tensor("msq0", [IPC, 128, 50], f32, kind="ExternalInput").ap()
    msq1 = nc.dram_tensor("msq1", [IPC, 100, 16], f32, kind="ExternalInput").ap()
    msq2 = nc.dram_tensor("msq2", [IPC, 100, 4], f32, kind="ExternalInput").ap()
    stats_d = nc.dram_tensor("stats", [128, 8], f32, kind="ExternalOutput").ap()

    with ExitStack() as ctx:
        tc = ctx.enter_context(tile.TileContext(nc))
        singles = ctx.enter_context(tc.tile_pool(name="singles", bufs=1))
        io = ctx.enter_context(tc.tile_pool(name="io", bufs=4))
        de = ctx.enter_context(tc.tile_pool(name="de", bufs=3))
        # one PSUM bank per unit (8 units = all 8 banks): psum tiles live
        # until the deferred mask-dot pass, so matmuls never wait on DVE
        ps_pool = ctx.enter_context(tc.tile_pool(name="ps_pool", bufs=8, space="PSUM"))

        ones_t = singles.tile([128, 1], bf16)
        nc.vector.memset(ones_t, 1.0)
        stats_t = singles.tile([128, 8], f32)
        nc.vector.memset(stats_t, 0.0)

        msq0_t = singles.tile([128, IPC, 50], f32)
        msq1_t = singles.tile([100, IPC, 16], f32)
        msq2_t = singles.tile([100, IPC, 4], f32)

        # two HWDGE rings (SP + ACT) — alternating halves the trigger-queue
        # fill time at the start and spreads steady-state trigger load
        dma_engines = [nc.sync, nc.scalar]
        dma_rr = [0]

        def dma(out, in_):
            eng = dma_engines[dma_rr[0] % 2]
            dma_rr[0] += 1
            eng.dma_start(out=out, in_=in_)

        def load_masks():
            nc.sync.dma_start(out=msq0_t[:], in_=msq0.rearrange("i p c -> p i c"))
            nc.sync.dma_start(out=msq1_t[:], in_=msq1.rearrange("i p c -> p i c"))
            nc.sync.dma_start(out=msq2_t[:], in_=msq2.rearrange("i p c -> p i c"))

        # deferred to a single pass at the end — interleaving these PE-gated
        # ops into the DVE stream stalls the in-order DVE between units
        mask_dots = []

        def mask_dot(ps, msq_ap, n_rows, col, uname):
            mask_dots.append((ps, msq_ap, n_rows, col))

        def run_mask_dots():
            for ps, msq_ap, n_rows, col in mask_dots:
                nc.vector.tensor_mul(ps[:], ps[:], msq_ap)  # in place in PSUM
                nc.vector.tensor_reduce(
                    out=stats_t[0:n_rows, col : col + 1],
                    in_=ps[:],
                    axis=(
                        mybir.AxisListType.X
                        if len(ps.shape) == 2
                        else mybir.AxisListType.XY
                    ),
                    op=Alu.add,
                )

        def alloc_pt(shape, uname):
            p_t = io.tile(shape, f32, tag="p", name=f"p_{uname}")
            t_t = io.tile(shape, f32, tag="t", name=f"t_{uname}")
            d_t = de.tile(shape, bf16, tag="d", name=f"d_{uname}")
            e_t = de.tile(shape, bf16, tag="e", name=f"e_{uname}")
            return p_t, t_t, d_t, e_t

        def sub_sq(p_t, t_t, d_t, e_t, sl):
            # fine-grained slices so compute trails the half-unit DMAs
            nc.vector.tensor_sub(d_t[sl], p_t[sl], t_t[sl])
            nc.scalar.square(e_t[sl], d_t[sl])

        def unit_l0(i, h, col):
            uname = f"l0_{i}_{h}"
            p_t, t_t, d_t, e_t = alloc_pt([128, 3200], uname)
            for q in range(2):  # two 1600-col half-DMAs per tensor
                sl = slice(h * 3200 + q * 1600, h * 3200 + (q + 1) * 1600)
                dst = (slice(None), slice(q * 1600, (q + 1) * 1600))
                dma(p_t[dst], p0[i, :, sl])
                dma(t_t[dst], t0[i, :, sl])
            for sb in range(4):  # 800-col compute blocks
                sub_sq(
                    p_t, t_t, d_t, e_t,
                    (slice(None), slice(sb * 800, (sb + 1) * 800)),
                )
            ps = ps_pool.tile([128, 25], f32, tag="ps", name=f"ps_{uname}")
            for j in range(25):
                nc.tensor.matmul(
                    ps[:, j : j + 1],
                    e_t[:, j * 128 : (j + 1) * 128],
                    ones_t[:, 0:1],
                    start=True,
                    stop=True,
                )
            mask_dot(ps, msq0_t[:, i, h * 25 : (h + 1) * 25], 128, col, uname)

        def unit_l0_split(i, h, col_a, col_b):
            # last unit: two independent 1600-col sub-units (12+13 chunks of
            # 128 px) so the post-DMA tail only depends on the second one
            uname = f"l0s_{i}_{h}"
            base = h * 3200
            for q, (ncols, nch, col) in enumerate(
                [(1536, 12, col_a), (1664, 13, col_b)]
            ):
                off = base + q * 1536
                p_t, t_t, d_t, e_t = alloc_pt([128, ncols], f"{uname}_{q}")
                half = ncols // 2
                for hh in range(2):
                    dst = (slice(None), slice(hh * half, (hh + 1) * half))
                    so = off + hh * half
                    dma(p_t[dst], p0[i, :, so : so + half])
                    dma(t_t[dst], t0[i, :, so : so + half])
                for sb in range(2):
                    sub_sq(
                        p_t, t_t, d_t, e_t,
                        (slice(None), slice(sb * half, (sb + 1) * half)),
                    )
                ps = ps_pool.tile([128, nch], f32, tag="ps", name=f"ps_{uname}_{q}")
                for j in range(nch):
                    nc.tensor.matmul(
                        ps[:, j : j + 1],
                        e_t[:, j * 128 : (j + 1) * 128],
                        ones_t[:, 0:1],
                        start=True,
                        stop=True,
                    )
                ch0 = h * 25 + q * 12
                mask_dot(ps, msq0_t[:, i, ch0 : ch0 + nch], 128, col, f"{uname}_{q}")

        def unit_l1(i, col):
            uname = f"l1_{i}"
            p_t, t_t, d_t, e_t = alloc_pt([128, 2, 1600], uname)
            psrc = p1[i].rearrange("(t p) x -> p t x", p=128)
            tsrc = t1[i].rearrange("(t p) x -> p t x", p=128)
            for t in range(2):  # one DMA per channel tile
                nc.sync.dma_start(out=p_t[:, t, :], in_=psrc[:, t, :])
                nc.sync.dma_start(out=t_t[:, t, :], in_=tsrc[:, t, :])
            for t in range(2):
                for q in range(2):
                    sub_sq(
                        p_t, t_t, d_t, e_t,
                        (slice(None), t, slice(q * 800, (q + 1) * 800)),
                    )
            ps = ps_pool.tile([100, 16], f32, tag="ps", name=f"ps_{uname}")
            for j in range(16):
                for t in range(2):
                    nc.tensor.matmul(
                        ps[:, j : j + 1],
                        e_t[:, t, j * 100 : (j + 1) * 100],
                        ones_t[:, 0:1],
                        start=(t == 0),
                        stop=(t == 1),
                    )
            mask_dot(ps, msq1_t[:, i, :], 100, col, uname)

        def unit_l2(col):
            uname = "l2"
            p_t, t_t, d_t, e_t = alloc_pt([128, IPC, 4, 400], uname)
            psrc = p2.rearrange("i (t p) x -> p i t x", p=128)
            tsrc = t2.rearrange("i (t p) x -> p i t x", p=128)
            for i in range(IPC):  # one DMA per image
                dma(p_t[:, i], psrc[:, i])
                dma(t_t[:, i], tsrc[:, i])
            for i in range(IPC):
                for q in range(2):
                    sub_sq(
                        p_t, t_t, d_t, e_t,
                        (slice(None), i, slice(q * 2, (q + 1) * 2), slice(None)),
                    )
            ps = ps_pool.tile([100, IPC, 4], f32, tag="ps", name=f"ps_{uname}")
            for i in range(IPC):
                for j in range(4):
                    for t in range(4):
                        nc.tensor.matmul(
                            ps[:, i, j : j + 1],
                            e_t[:, i, t, j * 100 : (j + 1) * 100],
                            ones_t[:, 0:1],
                            start=(t == 0),
                            stop=(t == 3),
                        )
            mask_dot(ps, msq2_t[:, :, :], 100, col, uname)

        # stats columns: 0-3 = level0 units, 4-5 = level1 units, 6 = level2.
        # Order: big/compute-heavy units early; a simple fine-grained L0
        # half-image last so the post-DMA tail is minimal.
        # NOTE: masks must be emitted before any consumer — Tile wires
        # dependencies in emission order.
        unit_l0(0, 0, 0)
        unit_l1(0, 4)
        unit_l0(0, 1, 1)
        unit_l2(6)
        unit_l0(1, 0, 2)
        unit_l1(1, 5)
        unit_l0_split(1, 1, 3, 7)
        load_masks()
        run_mask_dots()

        nc.sync.dma_start(out=stats_d, in_=stats_t[:])

    nc.compile()
    _PROG_CACHE["nc"] = nc
    return nc


# --------------------------------------------------------------------------
# host orchestration
# --------------------------------------------------------------------------
def make_in_maps(inputs, msq_levels):
    """Per-core input dicts. msq_levels: list of 3 arrays, m**2 per level
    arranged [B, n_chunks, chunk]."""
    m0, m1, m2 = msq_levels
    # [B, px] -> [B, p, chunk] with px = chunk*P + p
    a0 = np.ascontiguousarray(m0.reshape(B, 50, 128).transpose(0, 2, 1))
    a1 = np.ascontiguousarray(m1.reshape(B, 16, 100).transpose(0, 2, 1))
    a2 = np.ascontiguousarray(m2.reshape(B, 4, 100).transpose(0, 2, 1))
    names = ["y_pred0", "y_true0", "y_pred1", "y_true1", "y_pred2", "y_true2"]
    flat = {
        n: np.ascontiguousarray(np.asarray(inputs[n], dtype=np.float32)).reshape(
            B, LEVELS[int(n[-1])][0], -1
        )
        for n in names
    }
    in_maps = []
    for k in range(N_CORES):
        sl = slice(IPC * k, IPC * (k + 1))
        in_maps.append(
            {
                "p0": flat["y_pred0"][sl],
                "t0": flat["y_true0"][sl],
                "p1": flat["y_pred1"][sl],
                "t1": flat["y_true1"][sl],
                "p2": flat["y_pred2"][sl],
                "t2": flat["y_true2"][sl],
                "msq0": a0[sl],
                "msq1": a1[sl],
                "msq2": a2[sl],
            }
        )
    return in_maps


def combine(stats_list, npos):
    """stats_list: per-core [128, 8] partials. npos: [3] float64."""
    ssq = np.zeros(3, dtype=np.float64)
    for st in stats_list:
        st = np.asarray(st, dtype=np.float64)
        ssq[0] += st[:, 0:4].sum() + st[:, 7].sum()
        ssq[1] += st[:, 4:6].sum()
        ssq[2] += st[:, 6].sum()
    total = (ssq / npos).sum() / len(LEVELS)
    return np.float32(total)


def host_masks(inputs):
    bboxes = np.asarray(inputs["bboxes"], dtype=np.float32)
    batch_idx = np.asarray(inputs["batch_idx"], dtype=np.int32)
    msq_levels = []
    npos = np.zeros(3, dtype=np.float64)
    for li, (C, S) in enumerate(LEVELS):
        m = _gauss_mask_np(bboxes, batch_idx, S)  # [B, S, S]
        npos[li] = C * m.sum(dtype=np.float64)
        msq_levels.append((m.astype(np.float32) ** 2).reshape(B, S * S))
    return msq_levels, npos


def kernel(**inputs):
    global LAST_RESULTS
    import os

    from concourse.bass_utils import run_bass_kernel_spmd

    nc = build_program()
    msq_levels, npos = host_masks(inputs)
    in_maps = make_in_maps(inputs, msq_levels)
    trace = bool(int(os.environ.get("BOXGAUSS_TRACE", "0")))
    res = run_bass_kernel_spmd(nc, in_maps, list(range(N_CORES)), trace=trace)
    LAST_RESULTS = res
    return combine([r["stats"] for r in res.results], npos)


# revision 19
# speedup vs baseline: 1.0547x; 1.0547x over previous
"""Trainium2 (Bass/Tile) kernel for nn_BoxGauss: gaussian-box-masked MSE loss.

reference semantics (per pyramid level l with preds/trues [B, C, S, S]):
    m      = gauss_mask(bboxes, batch_idx, S, B)        # [B, S, S]
    n_pos  = C * sum(m)
    ssq    = sum((m[:, None] * (pred - true)) ** 2)
    total += ssq / n_pos
  output = total / n_levels                              # scalar f32

Strategy (data-parallel over 8 NeuronCores, 2 images per core):
  * The tiny mask m (built from 256 boxes) is computed on the host in
    fp32, mirroring the reference op-for-op; m**2 is shipped per-core,
    pre-arranged to the on-chip chunk layout (a few tens of KB).
  * Each core streams its 2 images of pred/true per level from HBM
    (~22.9 MB/core, the memory-bound bulk of the problem):
        DVE:  d = p - t                     (fp32 in, bf16 out)
        ACT:  e = d^2                       (bf16, Square is spline-exact)
        PE :  colsq[px_chunk] = ones^T-contraction over channels,
              i.e. matmul(lhsT=e[K=C_tile, M=px], rhs=ones[K,1]) -> PSUM
              accumulated over C tiles; pixels land on PSUM partitions.
              All units' columns share ONE [128, 140] PSUM bank.
        DVE:  one fused (psum * m^2) multiply + 3 per-level reduces.
  * Each core returns stats [128, 4]; host reduces the 8x tiny partials
    and applies the n_pos normalizers (all tiny scalar math).

Self-contained: shapes/sharding hardcoded for the
  y_pred0/1/2 [16,128,80,80]/[16,256,40,40]/[16,512,20,20] problem.
"""

import numpy as np

N_CORES = 8
B = 16
IPC = B // N_CORES  # images per core
STD = 2.0

# (C, S) per level
LEVELS = [(128, 80), (256, 40), (512, 20)]

_PROG_CACHE = {}
LAST_RESULTS = None  # BassKernelResults of the most recent device run


# --------------------------------------------------------------------------
# host-side mask (mirrors reference._gauss_mask in fp32 numpy)
# --------------------------------------------------------------------------
def _gauss_mask_np(bboxes, batch_idx, S):
    f32 = np.float32
    bb = np.asarray(bboxes, dtype=f32)
    g = np.floor(bb * f32(S)).astype(np.int32)
    xc, yc, w, h = g[:, 0], g[:, 1], g[:, 2], g[:, 3]
    xl = np.maximum(xc - w // 2, 0)
    xr = np.minimum(xc + w // 2, S - 1)
    yt = np.maximum(yc - h // 2, 0)
    yd = np.minimum(yc + h // 2, S - 1)
    width = (xr - xl + 1).astype(f32)
    height = (yd - yt + 1).astype(f32)
    ax = np.arange(S, dtype=f32)
    xcf = xc.astype(f32)
    ycf = yc.astype(f32)
    tx = (ax[None, :] - xcf[:, None]) ** 2 / (
        f32(STD * STD) * (width[:, None] / f32(2)) ** 2
    )
    ty = (ax[None, :] - ycf[:, None]) ** 2 / (
        f32(STD * STD) * (height[:, None] / f32(2)) ** 2
    )
    gauss = np.exp(-(tx[:, None, :] + ty[:, :, None]))  # [N, S, S] f32
    ix = (ax[None, :] >= xl[:, None]) & (ax[None, :] <= xr[:, None])
    iy = (ax[None, :] >= yt[:, None]) & (ax[None, :] <= yd[:, None])
    inbox = ix[:, None, :] & iy[:, :, None]
    gauss = np.where(inbox, gauss, f32(0))
    m = np.zeros((B, S, S), dtype=f32)
    bi = np.asarray(batch_idx)
    for n in range(bb.shape[0]):
        np.maximum(m[bi[n]], gauss[n], out=m[bi[n]])
    return m


# --------------------------------------------------------------------------
# device program (SPMD: same program on all 8 cores, per-core inputs)
# --------------------------------------------------------------------------
def build_program():
    if "nc" in _PROG_CACHE:
        return _PROG_CACHE["nc"]

    from contextlib import ExitStack

    import concourse.tile as tile
    from concourse import bacc, mybir

    f32 = mybir.dt.float32
    bf16 = mybir.dt.bfloat16
    Alu = mybir.AluOpType

    nc = bacc.Bacc("TRN2", target_bir_lowering=False, debug=False)

    p0 = nc.dram_tensor("p0", [IPC, 128, 6400], f32, kind="ExternalInput").ap()
    t0 = nc.dram_tensor("t0", [IPC, 128, 6400], f32, kind="ExternalInput").ap()
    p1 = nc.dram_tensor("p1", [IPC, 256, 1600], f32, kind="ExternalInput").ap()
    t1 = nc.dram_tensor("t1", [IPC, 256, 1600], f32, kind="ExternalInput").ap()
    p2 = nc.dram_tensor("p2", [IPC, 512, 400], f32, kind="ExternalInput").ap()
    t2 = nc.dram_tensor("t2", [IPC, 512, 400], f32, kind="ExternalInput").ap()
    msqall = nc.dram_tensor("msqall", [128, 140], f32, kind="ExternalInput").ap()
    stats_d = nc.dram_tensor("stats", [128, 4], f32, kind="ExternalOutput").ap()

    with ExitStack() as ctx:
        tc = ctx.enter_context(tile.TileContext(nc))
        singles = ctx.enter_context(tc.tile_pool(name="singles", bufs=1))
        io = ctx.enter_context(tc.tile_pool(name="io", bufs=4))
        de = ctx.enter_context(tc.tile_pool(name="de", bufs=3))
        # every unit's colsq columns fit in ONE psum bank ([128, 140] f32):
        # matmuls never wait on DVE; one fused mask-mul + 3 reduces at the end
        ps_pool = ctx.enter_context(tc.tile_pool(name="ps_pool", bufs=1, space="PSUM"))

        ones_t = singles.tile([128, 1], bf16)
        nc.vector.memset(ones_t, 1.0)
        stats_t = singles.tile([128, 4], f32)
        nc.vector.memset(stats_t, 0.0)
        msqall_t = singles.tile([128, 140], f32)
        ps_all = ps_pool.tile([128, 140], f32)
        # rows >= 100 of the l1/l2 columns are never written by the M=100
        # matmuls; zero the bank so mask-mul cannot hit NaN/Inf garbage
        nc.vector.memset(ps_all, 0.0)

        # two HWDGE rings (SP + ACT) — alternating halves the trigger-queue
        # fill time at the start and spreads steady-state trigger load
        dma_engines = [nc.sync, nc.scalar]
        dma_rr = [0]

        def dma(out, in_):
            eng = dma_engines[dma_rr[0] % 2]
            dma_rr[0] += 1
            eng.dma_start(out=out, in_=in_)

        def load_masks():
            nc.sync.dma_start(out=msqall_t[:], in_=msqall)

        def run_mask_dots():
            # one fused pass: weight all colsq columns, reduce per level
            nc.vector.tensor_mul(ps_all[:], ps_all[:], msqall_t[:])
            for li, (c0, c1) in enumerate([(0, 100), (100, 132), (132, 140)]):
                nc.vector.tensor_reduce(
                    out=stats_t[:, li : li + 1],
                    in_=ps_all[:, c0:c1],
                    axis=mybir.AxisListType.X,
                    op=Alu.add,
                )

        def alloc_pt(shape, uname):
            p_t = io.tile(shape, f32, tag="p", name=f"p_{uname}")
            t_t = io.tile(shape, f32, tag="t", name=f"t_{uname}")
            d_t = de.tile(shape, bf16, tag="d", name=f"d_{uname}")
            e_t = de.tile(shape, bf16, tag="e", name=f"e_{uname}")
            return p_t, t_t, d_t, e_t

        def sub_sq(p_t, t_t, d_t, e_t, sl):
            # fine-grained slices so compute trails the half-unit DMAs
            nc.vector.tensor_sub(d_t[sl], p_t[sl], t_t[sl])
            nc.scalar.square(e_t[sl], d_t[sl])

        def unit_l0(i, h, col):
            uname = f"l0_{i}_{h}"
            p_t, t_t, d_t, e_t = alloc_pt([128, 3200], uname)
            for q in range(2):  # two 1600-col half-DMAs per tensor
                sl = slice(h * 3200 + q * 1600, h * 3200 + (q + 1) * 1600)
                dst = (slice(None), slice(q * 1600, (q + 1) * 1600))
                dma(p_t[dst], p0[i, :, sl])
                dma(t_t[dst], t0[i, :, sl])
            for sb in range(4):  # 800-col compute blocks
                sub_sq(
                    p_t, t_t, d_t, e_t,
                    (slice(None), slice(sb * 800, (sb + 1) * 800)),
                )
            c0 = i * 50 + h * 25
            for j in range(25):
                nc.tensor.matmul(
                    ps_all[:, c0 + j : c0 + j + 1],
                    e_t[:, j * 128 : (j + 1) * 128],
                    ones_t[:, 0:1],
                    start=True,
                    stop=True,
                )

        def unit_l0_split(i, h, col_a, col_b):
            # last unit: two independent 1600-col sub-units (12+13 chunks of
            # 128 px) so the post-DMA tail only depends on the second one
            uname = f"l0s_{i}_{h}"
            base = h * 3200
            for q, (ncols, nch, col) in enumerate(
                [(1536, 12, col_a), (1664, 13, col_b)]
            ):
                off = base + q * 1536
                p_t, t_t, d_t, e_t = alloc_pt([128, ncols], f"{uname}_{q}")
                half = ncols // 2
                for hh in range(2):
                    dst = (slice(None), slice(hh * half, (hh + 1) * half))
                    so = off + hh * half
                    dma(p_t[dst], p0[i, :, so : so + half])
                    dma(t_t[dst], t0[i, :, so : so + half])
                for sb in range(2):
                    sub_sq(
                        p_t, t_t, d_t, e_t,
                        (slice(None), slice(sb * half, (sb + 1) * half)),
                    )
                c0 = i * 50 + h * 25 + q * 12
                for j in range(nch):
                    nc.tensor.matmul(
                        ps_all[:, c0 + j : c0 + j + 1],
                        e_t[:, j * 128 : (j + 1) * 128],
                        ones_t[:, 0:1],
                        start=True,
                        stop=True,
                    )

        def unit_l1(i, col):
            uname = f"l1_{i}"
            p_t, t_t, d_t, e_t = alloc_pt([128, 2, 1600], uname)
            psrc = p1[i].rearrange("(t p) x -> p t x", p=128)
            tsrc = t1[i].rearrange("(t p) x -> p t x", p=128)
            for t in range(2):  # one DMA per channel tile
                nc.sync.dma_start(out=p_t[:, t, :], in_=psrc[:, t, :])
                nc.sync.dma_start(out=t_t[:, t, :], in_=tsrc[:, t, :])
            for t in range(2):
                for q in range(2):
                    sub_sq(
                        p_t, t_t, d_t, e_t,
                        (slice(None), t, slice(q * 800, (q + 1) * 800)),
                    )
            c0 = 100 + i * 16
            for j in range(16):
                for t in range(2):
                    nc.tensor.matmul(
                        ps_all[0:100, c0 + j : c0 + j + 1],
                        e_t[:, t, j * 100 : (j + 1) * 100],
                        ones_t[:, 0:1],
                        start=(t == 0),
                        stop=(t == 1),
                    )

        def unit_l2(col):
            uname = "l2"
            p_t, t_t, d_t, e_t = alloc_pt([128, IPC, 4, 400], uname)
            psrc = p2.rearrange("i (t p) x -> p i t x", p=128)
            tsrc = t2.rearrange("i (t p) x -> p i t x", p=128)
            for i in range(IPC):  # one DMA per image
                dma(p_t[:, i], psrc[:, i])
                dma(t_t[:, i], tsrc[:, i])
            for i in range(IPC):
                for q in range(2):
                    sub_sq(
                        p_t, t_t, d_t, e_t,
                        (slice(None), i, slice(q * 2, (q + 1) * 2), slice(None)),
                    )
            for i in range(IPC):
                for j in range(4):
                    nc0 = 132 + i * 4 + j
                    for t in range(4):
                        nc.tensor.matmul(
                            ps_all[0:100, nc0 : nc0 + 1],
                            e_t[:, i, t, j * 100 : (j + 1) * 100],
                            ones_t[:, 0:1],
                            start=(t == 0),
                            stop=(t == 3),
                        )

        # stats columns: 0-3 = level0 units, 4-5 = level1 units, 6 = level2.
        # Order: big/compute-heavy units early; a simple fine-grained L0
        # half-image last so the post-DMA tail is minimal.
        # NOTE: masks must be emitted before any consumer — Tile wires
        # dependencies in emission order.
        unit_l0(0, 0, 0)
        unit_l1(0, 4)
        unit_l0(0, 1, 1)
        unit_l2(6)
        unit_l0(1, 0, 2)
        unit_l1(1, 5)
        unit_l0_split(1, 1, 3, 7)
        load_masks()
        run_mask_dots()

        nc.sync.dma_start(out=stats_d, in_=stats_t[:])

    nc.compile()
    _PROG_CACHE["nc"] = nc
    return nc


# --------------------------------------------------------------------------
# host orchestration
# --------------------------------------------------------------------------
def make_msqall(msq_levels):
    """[B, 128, 140] per-image mask-squared columns, matching the device
    psum column map: l0 image-half chunks 0-99, l1 100-131, l2 132-139."""
    m0, m1, m2 = msq_levels
    out = np.zeros((B, 128, 140), dtype=np.float32)
    out[:, :, 0:50] = m0.reshape(B, 50, 128).transpose(0, 2, 1)
    out[:, :100, 100:116] = m1.reshape(B, 16, 100).transpose(0, 2, 1)
    out[:, :100, 132:136] = m2.reshape(B, 4, 100).transpose(0, 2, 1)
    return out


def make_in_maps(inputs, msq_levels):
    """Per-core input dicts."""
    ma = make_msqall(msq_levels)
    names = ["y_pred0", "y_true0", "y_pred1", "y_true1", "y_pred2", "y_true2"]
    # fold each core's second image into the image-1 column slots

    flat = {
        n: np.ascontiguousarray(np.asarray(inputs[n], dtype=np.float32)).reshape(
            B, LEVELS[int(n[-1])][0], -1
        )
        for n in names
    }
    in_maps = []
    for k in range(N_CORES):
        sl = slice(IPC * k, IPC * (k + 1))
        mc = ma[sl].copy()  # [2, 128, 140]
        msq_core = np.zeros((128, 140), np.float32)
        msq_core[:, 0:50] = mc[0, :, 0:50]
        msq_core[:, 50:100] = mc[1, :, 0:50]
        msq_core[:, 100:116] = mc[0, :, 100:116]
        msq_core[:, 116:132] = mc[1, :, 100:116]
        msq_core[:, 132:136] = mc[0, :, 132:136]
        msq_core[:, 136:140] = mc[1, :, 132:136]
        in_maps.append(
            {
                "p0": flat["y_pred0"][sl],
                "t0": flat["y_true0"][sl],
                "p1": flat["y_pred1"][sl],
                "t1": flat["y_true1"][sl],
                "p2": flat["y_pred2"][sl],
                "t2": flat["y_true2"][sl],
                "msqall": np.ascontiguousarray(msq_core),
            }
        )
    return in_maps


def combine(stats_list, npos):
    """stats_list: per-core [128, 8] partials. npos: [3] float64."""
    ssq = np.zeros(3, dtype=np.float64)
    for st in stats_list:
        st = np.asarray(st, dtype=np.float64)
        for li in range(3):
            ssq[li] += st[:, li].sum()
    total = (ssq / npos).sum() / len(LEVELS)
    return np.float32(total)


def host_masks(inputs):
    bboxes = np.asarray(inputs["bboxes"], dtype=np.float32)
    batch_idx = np.asarray(inputs["batch_idx"], dtype=np.int32)
    msq_levels = []
    npos = np.zeros(3, dtype=np.float64)
    for li, (C, S) in enumerate(LEVELS):
        m = _gauss_mask_np(bboxes, batch_idx, S)  # [B, S, S]
        npos[li] = C * m.sum(dtype=np.float64)
        msq_levels.append((m.astype(np.float32) ** 2).reshape(B, S * S))
    return msq_levels, npos


def kernel(**inputs):
    global LAST_RESULTS
    import os

    from concourse.bass_utils import run_bass_kernel_spmd

    nc = build_program()
    msq_levels, npos = host_masks(inputs)
    in_maps = make_in_maps(inputs, msq_levels)
    trace = bool(int(os.environ.get("BOXGAUSS_TRACE", "0")))
    res = run_bass_kernel_spmd(nc, in_maps, list(range(N_CORES)), trace=trace)
    LAST_RESULTS = res
    return combine([r["stats"] for r in res.results], npos)
